# revision 12
# baseline (speedup 1.0000x reference)
"""Block-diagonal (per-frame) multi-head attention on 8 Trainium2 cores.

Problem: x[2,3200,512] -> QKV proj (H=8 heads, D=64) -> attention masked to
25-token frames (128 frames) -> out[2,3200,512].  N = 3200 = 128*25.

Sharding: 256 (batch, frame) groups; core c handles batch c//4, frames
(c%4)*32..+32  => 800 tokens/core, tiled as 8 x 100 tokens (4 frames).

Layout trick: host sends x pre-transposed (xT [512, 800]) so every matmul
contracts over the partition dim:
  qT/kT [feat, tok] = W.T @ xT   (lhsT = W slice, rhs = xT)
  v     [tok, feat] = xT.T @ Wv  (lhsT = xT slice, rhs = Wv)
Scores per (head, tile): S^T = kT_h.T @ qT_h directly, no transposes.
Projections run in f32r (1 cyc/row on PE since N>=256); scores/PV in f16
(1 cyc/row, ~11-bit mantissa).  The frame mask is applied MULTIPLICATIVELY
on the DVE after exp (exact 0/1 in f16, one [100,400] multiply per unit),
so no PE mask matmuls and the softmax skips max-subtraction (max
tile-local score ~5.7, exp~304 << f16 max).  The ones-column of v turns
PV's last column into the softmax denominator; the final per-head scale
is one broadcast DVE multiply.

DMA strategy: descriptor generation costs ~660ns of engine time per
dma_start (descriptors then spray across all 16 queues), so transfers are
packed into FEW large dma_starts using rearranged APs ([512,N] DRAM ->
[128, 4*N] SBUF in halves), alternated between the Sync and Scalar HWDGE
queues so issue overlaps; output stores issue from the otherwise-idle
GpSimd software DGE.  Attention units are software-pipelined two deep so
the PE isn't gated on Scalar's EXP latency.
"""

import numpy as np

B, N, DIN = 2, 3200, 512
H, D = 8, 64
TL, JN = 128, 25
NCORES = 8
TOK = 800      # tokens per core
NT = 8         # token tiles per core
TT = 100       # tokens per tile (4 frames)

# matmul dtype per stage: 'f32' | 'f32r' | 'bf16' | 'f16'
CONFIG = {"proj": "f16", "qk": "f16", "pv": "f16"}
LOOKAHEAD = 2   # attention software-pipeline depth

_CACHE = {}
LAST_RESULT = None  # BassKernelResults of the most recent kernel() call


def _build(cfg):
    import concourse.bacc as bacc
    import concourse.tile as tile
    from concourse import mybir

    f32 = mybir.dt.float32
    bf16 = mybir.dt.bfloat16
    f16 = mybir.dt.float16
    f32r = mybir.dt.float32r
    AF = mybir.ActivationFunctionType
    ALU = mybir.AluOpType

    def io_dt(kind):
        return {"f32": f32, "f32r": f32r, "bf16": bf16, "f16": f16}[kind]

    proj_dt, qk_dt, pv_dt = cfg["proj"], cfg["qk"], cfg["pv"]

    nc = bacc.Bacc("TRN2", target_bir_lowering=False, debug=False,
                   num_devices=NCORES)

    xt_d = nc.dram_tensor("xT", [DIN, TOK], io_dt(proj_dt),
                          kind="ExternalInput").ap()
    w_d = {}
    for nm in ("wq", "wk", "wv"):
        w_d[nm] = nc.dram_tensor(nm, [DIN, DIN], io_dt(proj_dt),
                                 kind="ExternalInput").ap()
    # bias_all: cols 0:4 = bq (scaled, folded per-ft), 4:8 = bk, 8:520 = bv
    ball_d = nc.dram_tensor("ball", [128, 8 + DIN], f32,
                            kind="ExternalInput").ap()
    m4_d = nc.dram_tensor("m4", [TT, 4 * TT], io_dt(pv_dt),
                          kind="ExternalInput").ap()
    out_d = nc.dram_tensor("out", [TOK, DIN], f16,
                          kind="ExternalOutput").ap()

    with tile.TileContext(nc) as tc:
        with (
            tc.tile_pool(name="persist", bufs=1) as pp,
            tc.tile_pool(name="scratch", bufs=2) as sp,
        ):
            # ---- SBUF tiles: one wide tile per tensor, k-chunks in free dim
            xt = pp.tile([128, 4 * TOK], io_dt(proj_dt), name="xt", tag="xt")
            wq = pp.tile([128, 4 * DIN], io_dt(proj_dt), name="wq", tag="wq")
            wk = pp.tile([128, 4 * DIN], io_dt(proj_dt), name="wk", tag="wk")
            wv = pp.tile([128, 4 * DIN], io_dt(proj_dt), name="wv", tag="wv")
            ball = pp.tile([128, 8 + DIN], f32, name="ball", tag="ball")
            m4 = pp.tile([TT, 4 * TT], io_dt(pv_dt), name="m4", tag="m4")

            # ---- DMA in: halves (k=0,1 | k=2,3) ordered by first use,
            # alternating between the sync and scalar HWDGE queues
            def load_half(eng, dst, src_d, width, c):
                src = src_d[c * 256:(c + 1) * 256, :].rearrange(
                    "(a p) f -> p a f", p=128)
                dst3 = dst[:, c * 2 * width:(c + 1) * 2 * width].rearrange(
                    "p (a f) -> p a f", f=width)
                eng.dma_start(out=dst3, in_=src)

            load_half(nc.sync, xt, xt_d, TOK, 0)
            load_half(nc.scalar, wq, w_d["wq"], DIN, 0)
            load_half(nc.sync, xt, xt_d, TOK, 1)
            load_half(nc.scalar, wq, w_d["wq"], DIN, 1)
            load_half(nc.sync, wk, w_d["wk"], DIN, 0)
            load_half(nc.scalar, wk, w_d["wk"], DIN, 1)
            nc.sync.dma_start(out=ball, in_=ball_d)
            nc.scalar.dma_start(out=m4, in_=m4_d)
            load_half(nc.sync, wv, w_d["wv"], DIN, 0)
            load_half(nc.scalar, wv, w_d["wv"], DIN, 1)

            bqc = ball[:, 0:4]
            bkc = ball[:, 4:8]
            bvb = ball[:, 8:8 + DIN]

            # ---- persistent activations ----
            qt = [pp.tile([128, TOK], io_dt(qk_dt), name=f"qt{k}",
                          tag=f"qt{k}") for k in range(4)]
            kt_ = [pp.tile([128, TOK], io_dt(qk_dt), name=f"kt{k}",
                           tag=f"kt{k}") for k in range(4)]
            # v with 65 columns per head: col h*65+64 is all-ones so the PV
            # matmul also produces the softmax denominator in its last column
            vt = [pp.tile([TT, H * (D + 1)], io_dt(pv_dt), name=f"vt{t}",
                          tag=f"vt{t}") for t in range(NT)]
            ot = [pp.tile([TT, DIN], f16, name=f"ot{t}", tag=f"ot{t}")
                  for t in range(NT)]

            with (
                tc.tile_pool(name="ppsum", bufs=3, space="PSUM") as pps,
                tc.tile_pool(name="vpsum", bufs=4, space="PSUM") as vps,
            ):
                # ---- q^T / k^T projections: psum[feat, tok] ----
                for (w, bc, dst) in ((wq, bqc, qt), (wk, bkc, kt_)):
                    for ft in range(4):
                        for ch in range(2):
                            csl = slice(ch * 400, (ch + 1) * 400)
                            acc = pps.tile([128, 400], f32, name="pacc",
                                           tag="p", bufs=3)
                            for k in range(4):
                                nc.tensor.matmul(
                                    acc[:],
                                    w[:, k * DIN + ft * 128:
                                      k * DIN + (ft + 1) * 128],
                                    xt[:, k * TOK + ch * 400:
                                       k * TOK + (ch + 1) * 400],
                                    start=(k == 0), stop=(k == 3))
                            nc.scalar.activation(dst[ft][:, csl], acc[:],
                                                 AF.Identity,
                                                 bias=bc[:, ft:ft + 1])

                # ---- v projection: psum[tok, feat]; bias+relu on DVE ----
                for t in range(NT):
                    acc = vps.tile([TT, DIN], f32, name="vacc", tag="v",
                                   bufs=4)
                    for k in range(4):
                        nc.tensor.matmul(
                            acc[:],
                            xt[:, k * TOK + t * TT:k * TOK + (t + 1) * TT],
                            wv[:, k * DIN:(k + 1) * DIN],
                            start=(k == 0), stop=(k == 3))
                    vdat = vt[t].rearrange("p (h c) -> p h c", c=D + 1)[:, :, :D]
                    vones = vt[t].rearrange("p (h c) -> p h c",
                                            c=D + 1)[:, :, D:D + 1]
                    nc.vector.scalar_tensor_tensor(
                        vdat, acc.rearrange("p (h c) -> p h c", c=D), 0.0,
                        bvb[:TT, :].rearrange("p (h c) -> p h c", c=D),
                        op0=ALU.add, op1=ALU.add)
                    nc.vector.tensor_scalar_max(vdat, vdat, 0.0)
                    nc.vector.memset(vones, 1.0)

            # ---- attention ----
            # Per (tile, head-group-of-4) unit: S^T = kT_h.T @ qT_h per head
            # into stE (even heads, base partition 0) / stO (odd heads, bp64)
            # -- separate PSUM banks so the PE's row-group-concurrent matmuls
            # never co-write a bank.  E = exp(S^T) on Scalar -> f16 halves of
            # one [100,400] tile, frame mask applied multiplicatively on DVE
            # in one op, PV with the ones-column denominator trick, then one
            # broadcast DVE scale.  Units are software-pipelined LOOKAHEAD
            # deep so the PE isn't gated on Scalar's EXP latency.
            with tc.tile_pool(name="apsum", bufs=2, space="PSUM") as aps:
                units = [(t, hg) for t in range(NT) for hg in range(2)]
                et_of = {}

                def stage1(u):
                    t, hg = u
                    tsl = slice(t * TT, (t + 1) * TT)
                    heads = [hg * 4, hg * 4 + 1, hg * 4 + 2, hg * 4 + 3]
                    # one 2-bank psum tile: even heads (bp0) in bank A cols
                    # 0:200, odd heads (bp64) in bank B cols 512:712 -- banks
                    # stay row-group separated, but ONE strided exp covers
                    # both
                    st = aps.tile([TT, 1024], f32, name="st", tag="s",
                                  bufs=3)
                    for i, h in enumerate(heads):
                        ft, po = h // 2, (h % 2) * 64
                        col = (h % 2) * 512 + (i // 2) * TT
                        nc.tensor.matmul(
                            st[:, col:col + TT],
                            kt_[ft][po:po + 64, tsl],
                            qt[ft][po:po + 64, tsl],
                            start=True, stop=True,
                            skip_group_check=True)
                    et = sp.tile([TT, 4 * TT], io_dt(pv_dt), name="et",
                                 tag="et", bufs=4)
                    stv = st[:].rearrange("p (b c) -> p b c", b=2)[:, :, 0:2 * TT]
                    etv = et[:].rearrange("p (b c) -> p b c", b=2)
                    nc.scalar.activation(etv, stv, AF.Exp)
                    nc.vector.scalar_tensor_tensor(et[:], et[:], 0.0,
                                                   m4[:], op0=ALU.add,
                                                   op1=ALU.mult)
                    et_of[u] = et

                def stage2(u):
                    t, hg = u
                    heads = [hg * 4, hg * 4 + 1, hg * 4 + 2, hg * 4 + 3]
                    et = et_of.pop(u)
                    # all 4 PVs of this unit share one PSUM bank (all their
                    # matmuls use rows 0-99 -> serialized in order, safe)
                    pv4 = aps.tile([TT, 4 * (D + 1)], f32, name="pv4",
                                   tag="pv", bufs=2)
                    for i, h in enumerate(heads):
                        col = (h % 2) * 2 * TT + (i // 2) * TT
                        nc.tensor.matmul(pv4[:, i * (D + 1):
                                             (i + 1) * (D + 1)],
                                         et[:, col:col + TT],
                                         vt[t][:, h * (D + 1):
                                               (h + 1) * (D + 1)],
                                         start=True, stop=True,
                                         skip_group_check=True)
                    pv4v = pv4.rearrange("p (h c) -> p h c", c=D + 1)
                    rc4 = sp.tile([TT, 4], f32, name="rc4", tag="rc", bufs=8)
                    nc.vector.reciprocal(rc4[:], pv4v[:, :, D:D + 1])
                    oview = ot[t][:, hg * 4 * D:(hg + 1) * 4 * D].rearrange(
                        "p (h c) -> p h c", c=D)
                    rcb = rc4[:].unsqueeze(2).broadcast_to((TT, 4, D))
                    nc.vector.scalar_tensor_tensor(
                        oview, pv4v[:, :, :D], 0.0, rcb,
                        op0=ALU.add, op1=ALU.mult)
                    tsl = slice(t * TT, (t + 1) * TT)
                    hsl = slice(hg * 256, (hg + 1) * 256)
                    eng = nc.sync if t == NT - 1 else nc.gpsimd
                    eng.dma_start(out=out_d[tsl, hsl], in_=ot[t][:, hsl])

                for i, u in enumerate(units):
                    stage1(u)
                    if i >= LOOKAHEAD:
                        stage2(units[i - LOOKAHEAD])
                for u in units[-LOOKAHEAD:]:
                    stage2(u)

    nc.compile()
    return nc


def _prep_inputs(x, Wq, bq, Wk, bk, Wv, bv, cfg):
    import ml_dtypes

    x = np.asarray(x, np.float32)
    Wq = np.asarray(Wq, np.float32)
    bq = np.asarray(bq, np.float32)
    Wk = np.asarray(Wk, np.float32)
    bk = np.asarray(bk, np.float32)
    Wv = np.asarray(Wv, np.float32)
    bv = np.asarray(bv, np.float32)

    def np_dt(kind):
        return {"bf16": ml_dtypes.bfloat16,
                "f16": np.float16}.get(kind, np.float32)

    scale = 1.0 / np.sqrt(np.float32(D))  # 1/8, exact
    wq_s = (Wq * scale).astype(np.float32)
    bq_s = (bq * scale).astype(np.float32)

    io_np = np_dt(cfg["proj"])
    xT = np.ascontiguousarray(x.transpose(0, 2, 1))  # [B, DIN, N]

    ball = np.empty((128, 8 + DIN), np.float32)
    ball[:, 0:4] = bq_s.reshape(4, 128).T
    ball[:, 4:8] = bk.reshape(4, 128).T
    ball[:, 8:] = np.tile(bv[None, :], (128, 1))

    # multiplicative frame mask over one 100-token tile, tiled 4x
    m1 = np.kron(np.eye(4, dtype=np.float32),
                 np.ones((JN, JN), np.float32))
    m4 = np.ascontiguousarray(np.tile(m1, (1, 4))).astype(np_dt(cfg["pv"]))

    in_maps = []
    for c in range(NCORES):
        b, fb = c // 4, c % 4
        in_maps.append({
            "xT": np.ascontiguousarray(
                xT[b, :, fb * TOK:(fb + 1) * TOK]).astype(io_np),
            "wq": wq_s.astype(io_np),
            "wk": Wk.astype(io_np),
            "wv": Wv.astype(io_np),
            "ball": ball,
            "m4": m4,
        })
    return in_maps


def kernel(x, Wq, bq, Wk, bk, Wv, bv, att_heads=H, latent_dim=D,
           time_len=TL, joint_num=JN, **_):
    from concourse.bass_utils import run_bass_kernel_spmd

    cfg = tuple(sorted(CONFIG.items()))
    if cfg not in _CACHE:
        _CACHE[cfg] = _build(CONFIG)
    nc = _CACHE[cfg]

    in_maps = _prep_inputs(x, Wq, bq, Wk, bk, Wv, bv, CONFIG)
    res = run_bass_kernel_spmd(nc, in_maps, core_ids=list(range(NCORES)))
    global LAST_RESULT
    LAST_RESULT = res

    out = np.empty((B, N, DIN), np.float32)
    for c in range(NCORES):
        b, fb = c // 4, c % 4
        out[b, fb * TOK:(fb + 1) * TOK, :] = res.results[c]["out"].astype(np.float32)
    return out


# revision 13
# speedup vs baseline: 1.0919x; 1.0919x over previous
"""Block-diagonal (per-frame) multi-head attention on 8 Trainium2 cores.

Problem: x[2,3200,512] -> QKV proj (H=8 heads, D=64) -> attention masked to
25-token frames (128 frames) -> out[2,3200,512].  N = 3200 = 128*25.

Sharding: 256 (batch, frame) groups; core c handles batch c//4, frames
(c%4)*32..+32  => 800 tokens/core, tiled as 8 x 100 tokens (4 frames).

Layout trick: host sends x pre-transposed (xT [512, 800]) so every matmul
contracts over the partition dim:
  qT/kT [feat, tok] = W.T @ xT   (lhsT = W slice, rhs = xT)
  v     [tok, feat] = xT.T @ Wv  (lhsT = xT slice, rhs = Wv)
Scores per (head, tile): S^T = kT_h.T @ qT_h directly, no transposes.
Projections run in f32r (1 cyc/row on PE since N>=256); scores/PV in f16
(1 cyc/row, ~11-bit mantissa).  The frame mask is applied MULTIPLICATIVELY
on the DVE after exp (exact 0/1 in f16, one [100,400] multiply per unit),
so no PE mask matmuls and the softmax skips max-subtraction (max
tile-local score ~5.7, exp~304 << f16 max).  The ones-column of v turns
PV's last column into the softmax denominator; the final per-head scale
is one broadcast DVE multiply.

DMA strategy: descriptor generation costs ~660ns of engine time per
dma_start (descriptors then spray across all 16 queues), so transfers are
packed into FEW large dma_starts using rearranged APs ([512,N] DRAM ->
[128, 4*N] SBUF in halves), alternated between the Sync and Scalar HWDGE
queues so issue overlaps; output stores issue from the otherwise-idle
GpSimd software DGE.  Attention units are software-pipelined two deep so
the PE isn't gated on Scalar's EXP latency.
"""

import numpy as np

B, N, DIN = 2, 3200, 512
H, D = 8, 64
TL, JN = 128, 25
NCORES = 8
TOK = 800      # tokens per core
NT = 8         # token tiles per core
TT = 100       # tokens per tile (4 frames)

# matmul dtype per stage: 'f32' | 'f32r' | 'bf16' | 'f16'
CONFIG = {"proj": "f16", "qk": "f16", "pv": "f16"}
LOOKAHEAD = 2   # attention software-pipeline depth

_CACHE = {}
LAST_RESULT = None  # BassKernelResults of the most recent kernel() call


def _build(cfg):
    import concourse.bacc as bacc
    import concourse.tile as tile
    from concourse import mybir

    f32 = mybir.dt.float32
    bf16 = mybir.dt.bfloat16
    f16 = mybir.dt.float16
    f32r = mybir.dt.float32r
    AF = mybir.ActivationFunctionType
    ALU = mybir.AluOpType

    def io_dt(kind):
        return {"f32": f32, "f32r": f32r, "bf16": bf16, "f16": f16}[kind]

    proj_dt, qk_dt, pv_dt = cfg["proj"], cfg["qk"], cfg["pv"]

    nc = bacc.Bacc("TRN2", target_bir_lowering=False, debug=False,
                   num_devices=NCORES)

    xt_d = nc.dram_tensor("xT", [DIN, TOK], io_dt(proj_dt),
                          kind="ExternalInput").ap()
    w_d = {}
    for nm in ("wq", "wk", "wv"):
        w_d[nm] = nc.dram_tensor(nm, [DIN, DIN], io_dt(proj_dt),
                                 kind="ExternalInput").ap()
    # bias_all: cols 0:4 = bq (scaled, folded per-ft), 4:8 = bk, 8:520 = bv
    ball_d = nc.dram_tensor("ball", [128, 8 + DIN], f32,
                            kind="ExternalInput").ap()
    m4_d = nc.dram_tensor("m4", [TT, 4 * TT], io_dt(pv_dt),
                          kind="ExternalInput").ap()
    out_d = nc.dram_tensor("out", [TOK, DIN], f16,
                          kind="ExternalOutput").ap()

    with tile.TileContext(nc) as tc:
        with (
            tc.tile_pool(name="persist", bufs=1) as pp,
            tc.tile_pool(name="scratch", bufs=2) as sp,
        ):
            # ---- SBUF tiles: one wide tile per tensor, k-chunks in free dim
            xt = pp.tile([128, 4 * TOK], io_dt(proj_dt), name="xt", tag="xt")
            wq = pp.tile([128, 4 * DIN], io_dt(proj_dt), name="wq", tag="wq")
            wk = pp.tile([128, 4 * DIN], io_dt(proj_dt), name="wk", tag="wk")
            wv = pp.tile([128, 4 * DIN], io_dt(proj_dt), name="wv", tag="wv")
            ball = pp.tile([128, 8 + DIN], f32, name="ball", tag="ball")
            m4 = pp.tile([TT, 4 * TT], io_dt(pv_dt), name="m4", tag="m4")

            # ---- DMA in: halves (k=0,1 | k=2,3) ordered by first use,
            # alternating between the sync and scalar HWDGE queues
            def load_half(eng, dst, src_d, width, c):
                src = src_d[c * 256:(c + 1) * 256, :].rearrange(
                    "(a p) f -> p a f", p=128)
                dst3 = dst[:, c * 2 * width:(c + 1) * 2 * width].rearrange(
                    "p (a f) -> p a f", f=width)
                eng.dma_start(out=dst3, in_=src)

            load_half(nc.sync, xt, xt_d, TOK, 0)
            load_half(nc.scalar, wq, w_d["wq"], DIN, 0)
            load_half(nc.sync, xt, xt_d, TOK, 1)
            load_half(nc.scalar, wq, w_d["wq"], DIN, 1)
            load_half(nc.sync, wk, w_d["wk"], DIN, 0)
            load_half(nc.scalar, wk, w_d["wk"], DIN, 1)
            nc.sync.dma_start(out=ball, in_=ball_d)
            nc.scalar.dma_start(out=m4, in_=m4_d)
            load_half(nc.sync, wv, w_d["wv"], DIN, 0)
            load_half(nc.scalar, wv, w_d["wv"], DIN, 1)

            bqc = ball[:, 0:4]
            bkc = ball[:, 4:8]
            bvb = ball[:, 8:8 + DIN]

            # ---- persistent activations ----
            qt = [pp.tile([128, TOK], io_dt(qk_dt), name=f"qt{k}",
                          tag=f"qt{k}") for k in range(4)]
            kt_ = [pp.tile([128, TOK], io_dt(qk_dt), name=f"kt{k}",
                           tag=f"kt{k}") for k in range(4)]
            # v with 65 columns per head: col h*65+64 is all-ones so the PV
            # matmul also produces the softmax denominator in its last column
            vt = [pp.tile([TT, H * (D + 1)], io_dt(pv_dt), name=f"vt{t}",
                          tag=f"vt{t}") for t in range(NT)]
            ot = [pp.tile([TT, DIN], f16, name=f"ot{t}", tag=f"ot{t}")
                  for t in range(NT)]

            with (
                tc.tile_pool(name="ppsum", bufs=3, space="PSUM") as pps,
                tc.tile_pool(name="vpsum", bufs=4, space="PSUM") as vps,
            ):
                # ---- q^T / k^T projections: psum[feat, tok] ----
                for (w, bc, dst) in ((wq, bqc, qt), (wk, bkc, kt_)):
                    for ft in range(4):
                        for ch in range(2):
                            csl = slice(ch * 400, (ch + 1) * 400)
                            acc = pps.tile([128, 400], f32, name="pacc",
                                           tag="p", bufs=3)
                            for k in range(4):
                                nc.tensor.matmul(
                                    acc[:],
                                    w[:, k * DIN + ft * 128:
                                      k * DIN + (ft + 1) * 128],
                                    xt[:, k * TOK + ch * 400:
                                       k * TOK + (ch + 1) * 400],
                                    start=(k == 0), stop=(k == 3))
                            nc.scalar.activation(dst[ft][:, csl], acc[:],
                                                 AF.Identity,
                                                 bias=bc[:, ft:ft + 1])

                # ---- v projection: psum[tok, feat]; bias+relu on DVE ----
                for t in range(NT):
                    acc = vps.tile([TT, DIN], f32, name="vacc", tag="v",
                                   bufs=4)
                    for k in range(4):
                        nc.tensor.matmul(
                            acc[:],
                            xt[:, k * TOK + t * TT:k * TOK + (t + 1) * TT],
                            wv[:, k * DIN:(k + 1) * DIN],
                            start=(k == 0), stop=(k == 3))
                    vdat = vt[t].rearrange("p (h c) -> p h c", c=D + 1)[:, :, :D]
                    vones = vt[t].rearrange("p (h c) -> p h c",
                                            c=D + 1)[:, :, D:D + 1]
                    nc.vector.scalar_tensor_tensor(
                        vdat, acc.rearrange("p (h c) -> p h c", c=D), 0.0,
                        bvb[:TT, :].rearrange("p (h c) -> p h c", c=D),
                        op0=ALU.add, op1=ALU.add)
                    nc.vector.tensor_scalar_max(vdat, vdat, 0.0)
                    nc.vector.memset(vones, 1.0)

            # ---- attention ----
            # Per (tile, head-group-of-4) unit: S^T = kT_h.T @ qT_h per head
            # into stE (even heads, base partition 0) / stO (odd heads, bp64)
            # -- separate PSUM banks so the PE's row-group-concurrent matmuls
            # never co-write a bank.  E = exp(S^T) on Scalar -> f16 halves of
            # one [100,400] tile, frame mask applied multiplicatively on DVE
            # in one op, PV with the ones-column denominator trick, then one
            # broadcast DVE scale.  Units are software-pipelined LOOKAHEAD
            # deep so the PE isn't gated on Scalar's EXP latency.
            with tc.tile_pool(name="apsum", bufs=2, space="PSUM") as aps:
                units = [(t, hg) for t in range(NT) for hg in range(2)]
                et_of = {}

                def stage1(u):
                    t, hg = u
                    tsl = slice(t * TT, (t + 1) * TT)
                    heads = [hg * 4, hg * 4 + 1, hg * 4 + 2, hg * 4 + 3]
                    stE = aps.tile([TT, 2 * TT], f32, name="stE", tag="s",
                                   bufs=6)
                    stO = aps.tile([TT, 2 * TT], f32, name="stO", tag="s",
                                   bufs=6)
                    for i, h in enumerate(heads):
                        ft, po = h // 2, (h % 2) * 64
                        dst = (stE, stO)[h % 2]
                        col = (i // 2) * TT
                        nc.tensor.matmul(
                            dst[:, col:col + TT],
                            kt_[ft][po:po + 64, tsl],
                            qt[ft][po:po + 64, tsl],
                            start=True, stop=True,
                            skip_group_check=True)
                    et = sp.tile([TT, 4 * TT], io_dt(pv_dt), name="et",
                                 tag="et", bufs=4)
                    nc.scalar.activation(et[:, 0:2 * TT], stE[:], AF.Exp)
                    nc.scalar.activation(et[:, 2 * TT:4 * TT], stO[:], AF.Exp)
                    nc.vector.scalar_tensor_tensor(et[:], et[:], 0.0,
                                                   m4[:], op0=ALU.add,
                                                   op1=ALU.mult)
                    et_of[u] = et

                def stage2(u):
                    t, hg = u
                    heads = [hg * 4, hg * 4 + 1, hg * 4 + 2, hg * 4 + 3]
                    et = et_of.pop(u)
                    # all 4 PVs of this unit share one PSUM bank (all their
                    # matmuls use rows 0-99 -> serialized in order, safe)
                    pv4 = aps.tile([TT, 4 * (D + 1)], f32, name="pv4",
                                   tag="pv", bufs=2)
                    for i, h in enumerate(heads):
                        col = (h % 2) * 2 * TT + (i // 2) * TT
                        nc.tensor.matmul(pv4[:, i * (D + 1):
                                             (i + 1) * (D + 1)],
                                         et[:, col:col + TT],
                                         vt[t][:, h * (D + 1):
                                               (h + 1) * (D + 1)],
                                         start=True, stop=True,
                                         skip_group_check=True)
                    pv4v = pv4.rearrange("p (h c) -> p h c", c=D + 1)
                    rc4 = sp.tile([TT, 4], f32, name="rc4", tag="rc", bufs=8)
                    nc.vector.reciprocal(rc4[:], pv4v[:, :, D:D + 1])
                    oview = ot[t][:, hg * 4 * D:(hg + 1) * 4 * D].rearrange(
                        "p (h c) -> p h c", c=D)
                    rcb = rc4[:].unsqueeze(2).broadcast_to((TT, 4, D))
                    nc.vector.scalar_tensor_tensor(
                        oview, pv4v[:, :, :D], 0.0, rcb,
                        op0=ALU.add, op1=ALU.mult)
                    tsl = slice(t * TT, (t + 1) * TT)
                    hsl = slice(hg * 256, (hg + 1) * 256)
                    eng = nc.sync if t == NT - 1 else nc.gpsimd
                    eng.dma_start(out=out_d[tsl, hsl], in_=ot[t][:, hsl])

                for i, u in enumerate(units):
                    stage1(u)
                    if i >= LOOKAHEAD:
                        stage2(units[i - LOOKAHEAD])
                for u in units[-LOOKAHEAD:]:
                    stage2(u)

    nc.compile()
    return nc


def _prep_inputs(x, Wq, bq, Wk, bk, Wv, bv, cfg):
    import ml_dtypes

    x = np.asarray(x, np.float32)
    Wq = np.asarray(Wq, np.float32)
    bq = np.asarray(bq, np.float32)
    Wk = np.asarray(Wk, np.float32)
    bk = np.asarray(bk, np.float32)
    Wv = np.asarray(Wv, np.float32)
    bv = np.asarray(bv, np.float32)

    def np_dt(kind):
        return {"bf16": ml_dtypes.bfloat16,
                "f16": np.float16}.get(kind, np.float32)

    scale = 1.0 / np.sqrt(np.float32(D))  # 1/8, exact
    wq_s = (Wq * scale).astype(np.float32)
    bq_s = (bq * scale).astype(np.float32)

    io_np = np_dt(cfg["proj"])
    xT = np.ascontiguousarray(x.transpose(0, 2, 1))  # [B, DIN, N]

    ball = np.empty((128, 8 + DIN), np.float32)
    ball[:, 0:4] = bq_s.reshape(4, 128).T
    ball[:, 4:8] = bk.reshape(4, 128).T
    ball[:, 8:] = np.tile(bv[None, :], (128, 1))

    # multiplicative frame mask over one 100-token tile, tiled 4x
    m1 = np.kron(np.eye(4, dtype=np.float32),
                 np.ones((JN, JN), np.float32))
    m4 = np.ascontiguousarray(np.tile(m1, (1, 4))).astype(np_dt(cfg["pv"]))

    in_maps = []
    for c in range(NCORES):
        b, fb = c // 4, c % 4
        in_maps.append({
            "xT": np.ascontiguousarray(
                xT[b, :, fb * TOK:(fb + 1) * TOK]).astype(io_np),
            "wq": wq_s.astype(io_np),
            "wk": Wk.astype(io_np),
            "wv": Wv.astype(io_np),
            "ball": ball,
            "m4": m4,
        })
    return in_maps


def kernel(x, Wq, bq, Wk, bk, Wv, bv, att_heads=H, latent_dim=D,
           time_len=TL, joint_num=JN, **_):
    from concourse.bass_utils import run_bass_kernel_spmd

    cfg = tuple(sorted(CONFIG.items()))
    if cfg not in _CACHE:
        _CACHE[cfg] = _build(CONFIG)
    nc = _CACHE[cfg]

    in_maps = _prep_inputs(x, Wq, bq, Wk, bk, Wv, bv, CONFIG)
    res = run_bass_kernel_spmd(nc, in_maps, core_ids=list(range(NCORES)))
    global LAST_RESULT
    LAST_RESULT = res

    out = np.empty((B, N, DIN), np.float32)
    for c in range(NCORES):
        b, fb = c // 4, c % 4
        out[b, fb * TOK:(fb + 1) * TOK, :] = res.results[c]["out"].astype(np.float32)
    return out
